# revision 2
# baseline (speedup 1.0000x reference)
"""Trainium2 Bass kernel v3 for the sparse_attention nn problem.

v3: all DVE ops use FLAT access patterns (strided 3D APs measured 5.4x
slower on HW). The unfold_h shift is folded into p2w on the host (shifted
d-slab layout with zero pad chunks); the chunk-shift moves to the t7
matmul's rhs AP (constant-stride 3-block read) and stationary pairing.

Differences vs baseline:
  - bf16 I/O: x, xT shipped bf16 from host; out returned bf16, upcast on host.
  - xT (p-major) is prepared on the host -> no PE transposes for x.
  - t6^T built by DVE partition-shifted adds from xT (roll is circular in p).
  - t2^T multiply runs 4x-mode (all-SBUF bf16).
  - Output stage: PSUM->SBUF bf16 copies on ACT, bf16 4x mult/add on DVE.

Math (per sample n):
  t2_d = p2w_d * shift_rows(x, 2(d-1));  t6 = x + roll(x,1,rows)
  t7[(d,c1),c] = sum_p t2_d[c1,p] t6[c,p] / 56
  t8full = W'' @ roll(x,1,rows),  W''[c,c'] = conv_w[c%4,c'] * p5w[c']
  t10[c,p] = sum_{d,c1} t7[(d,c1),c] t3_d[c1,p] / sqrt(384)
  out = t8full*x + t10
Scales folded into p2w on host: p2w' = p2w / (56*sqrt(384)).
Sharding: pure data parallel over batch (4 samples per core, 8 cores).
"""

import math
import numpy as np
import ml_dtypes

N, C, H, W, G = 32, 128, 56, 56, 32
HW = H * W                # 3136
NCORES = 8
NS = N // NCORES          # 4 samples per core
PCH = 2 * W               # 112 = p-chunk (2 image rows)
NCH = HW // PCH           # 28 chunks
FCH = 448                 # t10/t8 psum window (448*4B < 2KB psum bank)
NF = HW // FCH            # 7
SCALE = 1.0 / (56.0 * math.sqrt(384.0))
BF16NP = ml_dtypes.bfloat16

_CACHE = {}


def _body(tc, bass, mybir, xs, xts, t6ts, p2wt, w2t, ident, out, BF, F32, ctx, repeat=1, loop=1):
    nc = tc.nc
    mult = mybir.AluOpType.mult
    addop = mybir.AluOpType.add

    consts = ctx.enter_context(tc.tile_pool(name="consts", bufs=1))
    p_xbf = ctx.enter_context(tc.tile_pool(name="xbf", bufs=2))
    p_xt = ctx.enter_context(tc.tile_pool(name="xt", bufs=2))
    p_t3 = ctx.enter_context(tc.tile_pool(name="t3", bufs=2))
    p_t2a = ctx.enter_context(tc.tile_pool(name="t2a", bufs=1))
    p_t2b = ctx.enter_context(tc.tile_pool(name="t2b", bufs=1))
    p_t6t = ctx.enter_context(tc.tile_pool(name="t6t", bufs=2))
    p_t7sb = ctx.enter_context(tc.tile_pool(name="t7sb", bufs=2))
    p_fsb = ctx.enter_context(tc.tile_pool(name="fsb", bufs=3))
    p_out = ctx.enter_context(tc.tile_pool(name="outp", bufs=2))
    ps_t7 = ctx.enter_context(tc.tile_pool(name="pst7", bufs=2, space="PSUM"))
    ps_t7d = ctx.enter_context(tc.tile_pool(name="pst7d", bufs=1, space="PSUM"))
    ps_t10 = ctx.enter_context(tc.tile_pool(name="pst10", bufs=3, space="PSUM"))
    ps_t8 = ctx.enter_context(tc.tile_pool(name="pst8", bufs=2, space="PSUM"))

    p2wt_t = consts.tile([PCH, NCH * 3 * C], BF)
    nc.sync.dma_start(p2wt_t[:], p2wt)
    w2t_t = consts.tile([C, C], BF)
    nc.sync.dma_start(w2t_t[:], w2t)
    id_t = consts.tile([C, C], BF)
    nc.sync.dma_start(id_t[:], ident)

    # t2'^T in shifted d-slab layout with zero pads:
    # cols = [pad 128 | slab0 3584 | slab1 3584 | slab2 3584 | pad 128]
    # slab_d chunk j = p2w_d[chunk j-d+1] * x[chunk j]  (host pre-shifts p2w)
    # zero blocks: pad0, pad1, slab0 chunk 27, slab2 chunk 0 (both buffers)
    SLAB = NCH * C
    t2bufs = []
    for pool in (p_t2a, p_t2b):
        t2t_ = pool.tile([PCH, 2 * C + 3 * SLAB], BF, tag="t2t")
        nc.gpsimd.memset(t2t_[:, 0:C], 0.0)
        nc.gpsimd.memset(t2t_[:, C + (NCH - 1) * C : C + SLAB], 0.0)
        nc.gpsimd.memset(t2t_[:, C + 2 * SLAB : C + 2 * SLAB + C], 0.0)
        nc.gpsimd.memset(t2t_[:, C + 3 * SLAB : 2 * C + 3 * SLAB], 0.0)
        t2bufs.append(t2t_)

    def emit_samples():
        slist = [s for _ in range(repeat) for s in range(NS)]
        state = {}

        def prologue(s, idx):
            xbf = p_xbf.tile([C, HW], BF, tag="xbf")
            nc.sync.dma_start(xbf[:], xs[s])
            xt = p_xt.tile([PCH, NCH * C], BF, tag="xt")
            nc.sync.dma_start(xt[:], xts[s])

            # t3_0/t3_2: FLAT shifted copies + tiny strided zero memsets
            t30 = p_t3.tile([C, H, W], BF, tag="t30")
            t30f = t30[:].rearrange("c h w -> c (h w)")
            nc.vector.tensor_copy(t30f[:, 2:HW], xbf[:, 0 : HW - 2])
            nc.gpsimd.memset(t30[:, :, 0:2], 0.0)
            t32 = p_t3.tile([C, H, W], BF, tag="t32")
            t32f = t32[:].rearrange("c h w -> c (h w)")
            nc.vector.tensor_copy(t32f[:, 0 : HW - 2], xbf[:, 2:HW])
            nc.gpsimd.memset(t32[:, :, W - 2 : W], 0.0)

            # --- t2'^T = p2w'(shifted) * x^T: 3 FLAT DVE ops ---
            t2t = t2bufs[idx % 2]
            nc.vector.tensor_tensor(
                out=t2t[:, C : C + (NCH - 1) * C],
                in0=p2wt_t[:, 0 : (NCH - 1) * C],
                in1=xt[:, 0 : (NCH - 1) * C],
                op=mult,
            )
            nc.vector.tensor_tensor(
                out=t2t[:, C + SLAB : C + 2 * SLAB],
                in0=p2wt_t[:, SLAB : 2 * SLAB],
                in1=xt[:, :],
                op=mult,
            )
            nc.vector.tensor_tensor(
                out=t2t[:, C + 2 * SLAB + C : C + 3 * SLAB],
                in0=p2wt_t[:, 2 * SLAB + C : 3 * SLAB],
                in1=xt[:, C:SLAB],
                op=mult,
            )

            t6t = p_t6t.tile([PCH, NCH * C], BF, tag="t6t")
            nc.sync.dma_start(t6t[:], t6ts[s])

            # --- t7^T accumulation ---
            t2s = t2t[:].rearrange("p (b c) -> p b c", c=C)
            t7ps = ps_t7.tile([C, 3 * C], F32, tag="t7")
            for j in range(NCH):
                rhs = t2s[:, j : j + 2 * NCH + 3 : NCH + 1, :]
                nc.tensor.matmul(
                    t7ps[:],
                    t6t[:, j * C : (j + 1) * C],
                    rhs,
                    start=(j == 0),
                    stop=(j == NCH - 1),
                )
            t7T_sb = p_t7sb.tile([C, 3 * C], BF, tag="t7T")
            nc.vector.tensor_copy(t7T_sb[:], t7ps[:])
            t7dps = ps_t7d.tile([C, 3 * C], BF, tag="t7d")
            for d in range(3):
                nc.tensor.transpose(
                    t7dps[:, d * C : (d + 1) * C],
                    t7T_sb[:, d * C : (d + 1) * C],
                    id_t[:],
                )
            t7d_sb = p_t7sb.tile([C, 3 * C], BF, tag="t7d")
            nc.vector.tensor_copy(t7d_sb[:], t7dps[:])
            return dict(xbf=xbf, t30f=t30f, t32f=t32f, t7d_sb=t7d_sb)

        def windows(s, st):
            xbf, t30f, t32f, t7d_sb = st["xbf"], st["t30f"], st["t32f"], st["t7d_sb"]
            outsb = p_out.tile([C, HW], BF, tag="outp")
            t3list = (t30f, xbf[:], t32f)
            for f in range(NF):
                lo = f * FCH
                t8ps = ps_t8.tile([C, FCH], F32, tag="t8")
                if f == 0:
                    nc.tensor.matmul(
                        t8ps[:, 0:W], w2t_t[:], xbf[:, HW - W : HW],
                        start=True, stop=False,
                    )
                    nc.tensor.matmul(
                        t8ps[:, W:FCH], w2t_t[:], xbf[:, 0 : FCH - W],
                        start=False, stop=True,
                    )
                else:
                    nc.tensor.matmul(
                        t8ps[:], w2t_t[:], xbf[:, lo - W : lo + FCH - W],
                        start=True, stop=True,
                    )
                t10ps = ps_t10.tile([C, FCH], F32, tag="t10")
                for d in range(3):
                    nc.tensor.matmul(
                        t10ps[:],
                        t7d_sb[:, d * C : (d + 1) * C],
                        t3list[d][:, lo : lo + FCH],
                        start=(d == 0),
                        stop=(d == 2),
                    )
                t8sb = p_fsb.tile([C, FCH], BF, tag="t8sb")
                nc.scalar.copy(t8sb[:], t8ps[:])
                t10sb = p_fsb.tile([C, FCH], BF, tag="t10sb")
                nc.scalar.copy(t10sb[:], t10ps[:])
                t9sb = p_fsb.tile([C, FCH], BF, tag="t9sb")
                nc.vector.tensor_tensor(
                    out=t9sb[:], in0=t8sb[:], in1=xbf[:, lo : lo + FCH], op=mult
                )
                nc.vector.tensor_tensor(
                    out=outsb[:, lo : lo + FCH], in0=t9sb[:], in1=t10sb[:], op=addop
                )
            nc.sync.dma_start(out[s], outsb[:])

        # one-sample software pipeline: prologue(i+1) is emitted before windows(i)
        for i in range(len(slist) + 1):
            if i < len(slist):
                state[i] = prologue(slist[i], i)
            if i >= 1:
                windows(slist[i - 1], state.pop(i - 1))

    if loop > 1:
        with tc.For_i(0, loop, 1):
            emit_samples()
    else:
        emit_samples()


def build(repeat=1, loop=1):
    if ("nc", repeat, loop) in _CACHE:
        return _CACHE[("nc", repeat, loop)]
    from contextlib import ExitStack

    import concourse.bass as bass
    import concourse.tile as tile
    from concourse import bacc, mybir

    BF = mybir.dt.bfloat16
    F32 = mybir.dt.float32
    nc = bacc.Bacc("TRN2", target_bir_lowering=False, debug=False)
    xs = nc.dram_tensor("xs", [NS, C, HW], BF, kind="ExternalInput").ap()
    xts = nc.dram_tensor("xts", [NS, PCH, NCH * C], BF, kind="ExternalInput").ap()
    t6ts = nc.dram_tensor("t6ts", [NS, PCH, NCH * C], BF, kind="ExternalInput").ap()
    p2wt = nc.dram_tensor("p2wt", [PCH, NCH * 3 * C], BF, kind="ExternalInput").ap()
    w2t = nc.dram_tensor("w2t", [C, C], BF, kind="ExternalInput").ap()
    ident = nc.dram_tensor("ident", [C, C], BF, kind="ExternalInput").ap()
    out = nc.dram_tensor("out", [NS, C, HW], BF, kind="ExternalOutput").ap()

    with tile.TileContext(nc) as tc:
        with ExitStack() as ctx:
            _body(tc, bass, mybir, xs, xts, t6ts, p2wt, w2t, ident, out, BF, F32, ctx, repeat, loop)
    nc.compile()
    _CACHE[("nc", repeat, loop)] = nc
    return nc


def host_inputs(x, p2w, p5w, conv_w):
    """Shard + prep per-core input maps from full inputs."""
    x = np.asarray(x, dtype=np.float32).reshape(N, C, HW)
    xbf = np.ascontiguousarray(x).astype(BF16NP)
    # x^T chunks: xt[n, l, j*C + c] = x[n, c, 112j + l]
    xt = np.ascontiguousarray(
        x.reshape(N, C, NCH, PCH).transpose(0, 3, 2, 1)
    ).reshape(N, PCH, NCH * C).astype(BF16NP)
    # t6 = x + roll(x, 1, rows); bf16(x) summed in f32, rounded once to bf16
    xb32 = xbf.astype(np.float32).reshape(N, C, H, W)
    t6 = (xb32 + np.roll(xb32, 1, axis=2)).reshape(N, C, HW)
    t6t = np.ascontiguousarray(
        t6.reshape(N, C, NCH, PCH).transpose(0, 3, 2, 1)
    ).reshape(N, PCH, NCH * C).astype(BF16NP)
    p2w_ = (np.asarray(p2w, dtype=np.float32)[0] * SCALE).reshape(C, 3, HW)
    # shifted d-slab: p2wt[l, d, j, c1] = p2w[c1, d, 112*(j-d+1)+l] (else 0)
    pw = p2w_.transpose(1, 2, 0).reshape(3, NCH, PCH, C)   # (d, j, l, c1)
    slab = np.zeros((3, NCH, PCH, C), np.float32)
    slab[0, 0 : NCH - 1] = pw[0, 1:NCH]
    slab[1] = pw[1]
    slab[2, 1:NCH] = pw[2, 0 : NCH - 1]
    p2wt = np.ascontiguousarray(
        slab.transpose(2, 0, 1, 3).reshape(PCH, 3 * NCH * C)
    ).astype(BF16NP)
    p5 = np.asarray(p5w, dtype=np.float32).reshape(C)
    cw = np.asarray(conv_w, dtype=np.float32)        # (C//G, C)
    W2 = cw[np.arange(C) % (C // G)] * p5[None, :]   # (c, c')
    w2t = np.ascontiguousarray(W2.T).astype(BF16NP)  # (c', c)
    ident = np.eye(C, dtype=BF16NP)
    in_maps = [
        {
            "xs": np.ascontiguousarray(xbf[i * NS : (i + 1) * NS]),
            "xts": np.ascontiguousarray(xt[i * NS : (i + 1) * NS]),
            "t6ts": np.ascontiguousarray(t6t[i * NS : (i + 1) * NS]),
            "p2wt": p2wt,
            "w2t": w2t,
            "ident": ident,
        }
        for i in range(NCORES)
    ]
    return in_maps


def _get_runner(repeat=1, loop=1):
    """Build (once) a persistent jitted shard_map executable over 8 cores."""
    if ("runner", repeat, loop) in _CACHE:
        return _CACHE[("runner", repeat, loop)]
    import jax
    from jax.sharding import Mesh, PartitionSpec
    from jax.experimental.shard_map import shard_map
    from concourse import bass2jax, mybir

    nc = build(repeat, loop)
    bass2jax.install_neuronx_cc_hook()

    partition_name = nc.partition_id_tensor.name if nc.partition_id_tensor else None
    in_names, out_names, out_avals, zero_outs = [], [], [], []
    for alloc in nc.m.functions[0].allocations:
        if not isinstance(alloc, mybir.MemoryLocationSet):
            continue
        name = alloc.memorylocations[0].name
        if alloc.kind == "ExternalInput":
            if name != partition_name:
                in_names.append(name)
        elif alloc.kind == "ExternalOutput":
            shape = tuple(alloc.tensor_shape)
            dtype = mybir.dt.np(alloc.dtype)
            out_avals.append(jax.core.ShapedArray(shape, dtype))
            zero_outs.append(np.zeros(shape, dtype))
            out_names.append(name)
    n_params = len(in_names)
    n_outs = len(out_avals)
    all_in_names = list(in_names) + list(out_names)
    if partition_name is not None:
        all_in_names.append(partition_name)
    donate = tuple(range(n_params, n_params + n_outs))

    def _body(*args):
        operands = list(args)
        if partition_name is not None:
            operands.append(bass2jax.partition_id_tensor())
        outs = bass2jax._bass_exec_p.bind(
            *operands,
            out_avals=tuple(out_avals),
            in_names=tuple(all_in_names),
            out_names=tuple(out_names),
            lowering_input_output_aliases=(),
            sim_require_finite=True,
            sim_require_nnan=True,
            nc=nc,
        )
        return tuple(outs)

    devices = jax.devices()[:NCORES]
    mesh = Mesh(np.asarray(devices), ("core",))
    in_specs = (PartitionSpec("core"),) * (n_params + n_outs)
    out_specs = (PartitionSpec("core"),) * n_outs
    sharded = jax.jit(
        shard_map(
            _body, mesh=mesh, in_specs=in_specs, out_specs=out_specs, check_rep=False
        ),
        donate_argnums=donate,
        keep_unused=True,
    )
    runner = {
        "fn": sharded,
        "in_names": in_names,
        "out_names": out_names,
        "out_avals": out_avals,
        "mesh": mesh,
        "n_params": n_params,
    }
    _CACHE[("runner", repeat, loop)] = runner
    return runner


def _concat_inputs(runner, in_maps):
    return [
        np.concatenate([np.asarray(m[name]) for m in in_maps], axis=0)
        for name in runner["in_names"]
    ]


def _zero_bufs(runner):
    return [
        np.zeros((NCORES * a.shape[0], *a.shape[1:]), a.dtype)
        for a in runner["out_avals"]
    ]


def run_fast(in_maps):
    """Execute via the cached jitted executable; returns list of per-core dicts."""
    runner = _get_runner()
    out_arrs = runner["fn"](*_concat_inputs(runner, in_maps), *_zero_bufs(runner))
    res = []
    for c in range(NCORES):
        res.append(
            {
                name: np.asarray(out_arrs[i]).reshape(
                    NCORES, *runner["out_avals"][i].shape
                )[c]
                for i, name in enumerate(runner["out_names"])
            }
        )
    return res


def run(in_maps, trace=False, **kw):
    from concourse.bass_utils import run_bass_kernel_spmd

    nc = build()
    return run_bass_kernel_spmd(nc, in_maps, list(range(NCORES)), trace=trace, **kw)


def bench_repeat(in_maps, R=8, reps=10, iters=8):
    """Per-kernel time from an R-times-unrolled program vs the 1x program."""
    import time

    import jax
    from jax.sharding import NamedSharding, PartitionSpec

    def timed(repeat):
        runner = _get_runner(repeat)
        sh = NamedSharding(runner["mesh"], PartitionSpec("core"))
        dev_in = [jax.device_put(a, sh) for a in _concat_inputs(runner, in_maps)]
        outs = [jax.device_put(z, sh) for z in _zero_bufs(runner)]
        jax.block_until_ready(dev_in)
        jax.block_until_ready(outs)
        outs = runner["fn"](*dev_in, *outs)  # warmup + first chain
        jax.block_until_ready(outs)
        ts = []
        for _ in range(reps):
            t0 = time.perf_counter()
            for _ in range(iters):
                outs = runner["fn"](*dev_in, *outs)
            jax.block_until_ready(outs)
            ts.append((time.perf_counter() - t0) / iters)
        return min(ts)

    t1 = timed(1)
    tR = timed(R)
    per = (tR - t1) / (R - 1)
    return per, t1, tR


def kernel(x, p2w, p5w, conv_w):
    in_maps = host_inputs(x, p2w, p5w, conv_w)
    res = run_fast(in_maps)
    outs = [np.asarray(res[i]["out"]) for i in range(NCORES)]
    return (
        np.concatenate(outs, axis=0).reshape(N, C, H, W).astype(np.float32)
    )


def bench_loop(in_maps, K1=2, K2=18, rounds=12, iters=4):
    """Per-kernel time from the slope between two loop-count programs of
    identical size. Interleaved rounds cancel drift."""
    import time
    import jax
    from jax.sharding import NamedSharding, PartitionSpec

    state = {}
    for K in (K1, K2):
        r = _get_runner(1, K)
        sh = NamedSharding(r["mesh"], PartitionSpec("core"))
        dev_in = [jax.device_put(a, sh) for a in _concat_inputs(r, in_maps)]
        outs = [jax.device_put(z, sh) for z in _zero_bufs(r)]
        jax.block_until_ready(dev_in)
        jax.block_until_ready(outs)
        outs = r["fn"](*dev_in, *outs)
        jax.block_until_ready(outs)
        state[K] = [r, dev_in, outs, 1e9]
    for _ in range(rounds):
        for K in (K1, K2):
            r, dev_in, outs, best = state[K]
            t0 = time.perf_counter()
            for _ in range(iters):
                outs = r["fn"](*dev_in, *outs)
            jax.block_until_ready(outs)
            dt = (time.perf_counter() - t0) / iters
            state[K][2] = outs
            state[K][3] = min(best, dt)
    t1, t2 = state[K1][3], state[K2][3]
    return (t2 - t1) / (K2 - K1), t1, t2


# revision 3
# speedup vs baseline: 2.0646x; 2.0646x over previous
"""Trainium2 Bass kernel v3 for the sparse_attention nn problem.

v3: all DVE ops use FLAT access patterns (strided 3D APs measured 5.4x
slower on HW). The unfold_h shift is folded into p2w on the host (shifted
d-slab layout with zero pad chunks); the chunk-shift moves to the t7
matmul's rhs AP (constant-stride 3-block read) and stationary pairing.

Differences vs baseline:
  - bf16 I/O: x, xT shipped bf16 from host; out returned bf16, upcast on host.
  - xT (p-major) is prepared on the host -> no PE transposes for x.
  - t6^T built by DVE partition-shifted adds from xT (roll is circular in p).
  - t2^T multiply runs 4x-mode (all-SBUF bf16).
  - Output stage: PSUM->SBUF bf16 copies on ACT, bf16 4x mult/add on DVE.

Math (per sample n):
  t2_d = p2w_d * shift_rows(x, 2(d-1));  t6 = x + roll(x,1,rows)
  t7[(d,c1),c] = sum_p t2_d[c1,p] t6[c,p] / 56
  t8full = W'' @ roll(x,1,rows),  W''[c,c'] = conv_w[c%4,c'] * p5w[c']
  t10[c,p] = sum_{d,c1} t7[(d,c1),c] t3_d[c1,p] / sqrt(384)
  out = t8full*x + t10
Scales folded into p2w on host: p2w' = p2w / (56*sqrt(384)).
Sharding: pure data parallel over batch (4 samples per core, 8 cores).
"""

import math
import numpy as np
import ml_dtypes

N, C, H, W, G = 32, 128, 56, 56, 32
HW = H * W                # 3136
NCORES = 8
NS = N // NCORES          # 4 samples per core
PCH = 2 * W               # 112 = p-chunk (2 image rows)
NCH = HW // PCH           # 28 chunks
FCH = 448                 # t10/t8 psum window (448*4B < 2KB psum bank)
NF = HW // FCH            # 7
SCALE = 1.0 / (56.0 * math.sqrt(384.0))
BF16NP = ml_dtypes.bfloat16

_CACHE = {}


def _body(tc, bass, mybir, xs, xts, t6ts, p2wt, w2t, ident, out, BF, F32, ctx, repeat=1, loop=1):
    nc = tc.nc
    mult = mybir.AluOpType.mult
    addop = mybir.AluOpType.add

    consts = ctx.enter_context(tc.tile_pool(name="consts", bufs=1))
    p_xbf = ctx.enter_context(tc.tile_pool(name="xbf", bufs=2))
    p_xt = ctx.enter_context(tc.tile_pool(name="xt", bufs=2))
    p_t3 = ctx.enter_context(tc.tile_pool(name="t3", bufs=2))
    p_t2a = ctx.enter_context(tc.tile_pool(name="t2a", bufs=1))
    p_t2b = ctx.enter_context(tc.tile_pool(name="t2b", bufs=1))
    p_t6t = ctx.enter_context(tc.tile_pool(name="t6t", bufs=2))
    p_t7sb = ctx.enter_context(tc.tile_pool(name="t7sb", bufs=2))
    p_fsb = ctx.enter_context(tc.tile_pool(name="fsb", bufs=3))
    p_out = ctx.enter_context(tc.tile_pool(name="outp", bufs=2))
    ps_t7 = ctx.enter_context(tc.tile_pool(name="pst7", bufs=2, space="PSUM"))
    ps_t7d = ctx.enter_context(tc.tile_pool(name="pst7d", bufs=1, space="PSUM"))
    ps_cmb = ctx.enter_context(tc.tile_pool(name="pscmb", bufs=2, space="PSUM"))

    p2wt_t = consts.tile([PCH, NCH * 3 * C], BF)
    nc.sync.dma_start(p2wt_t[:], p2wt)
    w2t_t = consts.tile([C, C], BF)
    nc.sync.dma_start(w2t_t[:], w2t)
    id_t = consts.tile([C, C], BF)
    nc.sync.dma_start(id_t[:], ident)

    # t2'^T in shifted d-slab layout with zero pads:
    # cols = [pad 128 | slab0 3584 | slab1 3584 | slab2 3584 | pad 128]
    # slab_d chunk j = p2w_d[chunk j-d+1] * x[chunk j]  (host pre-shifts p2w)
    # zero blocks: pad0, pad1, slab0 chunk 27, slab2 chunk 0 (both buffers)
    SLAB = NCH * C
    t2bufs = []
    for pool in (p_t2a, p_t2b):
        t2t_ = pool.tile([PCH, 2 * C + 3 * SLAB], BF, tag="t2t")
        nc.gpsimd.memset(t2t_[:, 0:C], 0.0)
        nc.gpsimd.memset(t2t_[:, C + (NCH - 1) * C : C + SLAB], 0.0)
        nc.gpsimd.memset(t2t_[:, C + 2 * SLAB : C + 2 * SLAB + C], 0.0)
        nc.gpsimd.memset(t2t_[:, C + 3 * SLAB : 2 * C + 3 * SLAB], 0.0)
        t2bufs.append(t2t_)

    def emit_samples():
        slist = [s for _ in range(repeat) for s in range(NS)]
        state = {}

        def prologue(s, idx):
            xbf = p_xbf.tile([C, HW], BF, tag="xbf")
            nc.sync.dma_start(xbf[:], xs[s])
            xt = p_xt.tile([PCH, NCH * C], BF, tag="xt")
            nc.sync.dma_start(xt[:], xts[s])

            # t3_0/t3_2: FLAT shifted copies + tiny strided zero memsets
            t30 = p_t3.tile([C, H, W], BF, tag="t30")
            t30f = t30[:].rearrange("c h w -> c (h w)")
            nc.vector.tensor_copy(t30f[:, 2:HW], xbf[:, 0 : HW - 2])
            nc.gpsimd.memset(t30[:, :, 0:2], 0.0)
            t32 = p_t3.tile([C, H, W], BF, tag="t32")
            t32f = t32[:].rearrange("c h w -> c (h w)")
            nc.vector.tensor_copy(t32f[:, 0 : HW - 2], xbf[:, 2:HW])
            nc.gpsimd.memset(t32[:, :, W - 2 : W], 0.0)

            # --- t2'^T = p2w'(shifted) * x^T: 3 FLAT DVE ops ---
            t2t = t2bufs[idx % 2]
            nc.vector.tensor_tensor(
                out=t2t[:, C : C + (NCH - 1) * C],
                in0=p2wt_t[:, 0 : (NCH - 1) * C],
                in1=xt[:, 0 : (NCH - 1) * C],
                op=mult,
            )
            nc.vector.tensor_tensor(
                out=t2t[:, C + SLAB : C + 2 * SLAB],
                in0=p2wt_t[:, SLAB : 2 * SLAB],
                in1=xt[:, :],
                op=mult,
            )
            nc.vector.tensor_tensor(
                out=t2t[:, C + 2 * SLAB + C : C + 3 * SLAB],
                in0=p2wt_t[:, 2 * SLAB + C : 3 * SLAB],
                in1=xt[:, C:SLAB],
                op=mult,
            )

            t6t = p_t6t.tile([PCH, NCH * C], BF, tag="t6t")
            nc.sync.dma_start(t6t[:], t6ts[s])

            # --- t7^T accumulation ---
            t2s = t2t[:].rearrange("p (b c) -> p b c", c=C)
            t7ps = ps_t7.tile([C, 3 * C], F32, tag="t7")
            for j in range(NCH):
                rhs = t2s[:, j : j + 2 * NCH + 3 : NCH + 1, :]
                nc.tensor.matmul(
                    t7ps[:],
                    t6t[:, j * C : (j + 1) * C],
                    rhs,
                    start=(j == 0),
                    stop=(j == NCH - 1),
                )
            t7T_sb = p_t7sb.tile([C, 3 * C], BF, tag="t7T")
            nc.vector.tensor_copy(t7T_sb[:], t7ps[:])
            t7dps = ps_t7d.tile([C, 3 * C], BF, tag="t7d")
            for d in range(3):
                nc.tensor.transpose(
                    t7dps[:, d * C : (d + 1) * C],
                    t7T_sb[:, d * C : (d + 1) * C],
                    id_t[:],
                )
            t7d_sb = p_t7sb.tile([C, 3 * C], BF, tag="t7d")
            nc.vector.tensor_copy(t7d_sb[:], t7dps[:])
            return dict(xbf=xbf, t30f=t30f, t32f=t32f, t7d_sb=t7d_sb)

        def windows(s, st):
            xbf, t30f, t32f, t7d_sb = st["xbf"], st["t30f"], st["t32f"], st["t7d_sb"]
            outsb = p_out.tile([C, HW], BF, tag="outp")
            t3list = (t30f, xbf[:], t32f)
            for f in range(NF):
                lo = f * FCH
                cmb = ps_cmb.tile([C, 1024], F32, tag="cmb")
                t8ps = cmb[:, 0:FCH]
                if f == 0:
                    nc.tensor.matmul(
                        t8ps[:, 0:W], w2t_t[:], xbf[:, HW - W : HW],
                        start=True, stop=False,
                    )
                    nc.tensor.matmul(
                        t8ps[:, W:FCH], w2t_t[:], xbf[:, 0 : FCH - W],
                        start=False, stop=True,
                    )
                else:
                    nc.tensor.matmul(
                        t8ps[:], w2t_t[:], xbf[:, lo - W : lo + FCH - W],
                        start=True, stop=True,
                    )
                t10ps = cmb[:, 512 : 512 + FCH]
                for d in range(3):
                    nc.tensor.matmul(
                        t10ps[:],
                        t7d_sb[:, d * C : (d + 1) * C],
                        t3list[d][:, lo : lo + FCH],
                        start=(d == 0),
                        stop=(d == 2),
                    )
                csb = p_fsb.tile([C, 1024], BF, tag="csb")
                nc.scalar.copy(csb[:], cmb[:])
                t9sb = p_fsb.tile([C, FCH], BF, tag="t9sb")
                nc.vector.tensor_tensor(
                    out=t9sb[:], in0=csb[:, 0:FCH], in1=xbf[:, lo : lo + FCH], op=mult
                )
                nc.vector.tensor_tensor(
                    out=outsb[:, lo : lo + FCH],
                    in0=t9sb[:],
                    in1=csb[:, 512 : 512 + FCH],
                    op=addop,
                )
            nc.sync.dma_start(out[s], outsb[:])

        # one-sample software pipeline: prologue(i+1) is emitted before windows(i)
        for i in range(len(slist) + 1):
            if i < len(slist):
                state[i] = prologue(slist[i], i)
            if i >= 1:
                windows(slist[i - 1], state.pop(i - 1))

    if loop > 1:
        with tc.For_i(0, loop, 1):
            emit_samples()
    else:
        emit_samples()


def build(repeat=1, loop=1):
    if ("nc", repeat, loop) in _CACHE:
        return _CACHE[("nc", repeat, loop)]
    from contextlib import ExitStack

    import concourse.bass as bass
    import concourse.tile as tile
    from concourse import bacc, mybir

    BF = mybir.dt.bfloat16
    F32 = mybir.dt.float32
    nc = bacc.Bacc("TRN2", target_bir_lowering=False, debug=False)
    xs = nc.dram_tensor("xs", [NS, C, HW], BF, kind="ExternalInput").ap()
    xts = nc.dram_tensor("xts", [NS, PCH, NCH * C], BF, kind="ExternalInput").ap()
    t6ts = nc.dram_tensor("t6ts", [NS, PCH, NCH * C], BF, kind="ExternalInput").ap()
    p2wt = nc.dram_tensor("p2wt", [PCH, NCH * 3 * C], BF, kind="ExternalInput").ap()
    w2t = nc.dram_tensor("w2t", [C, C], BF, kind="ExternalInput").ap()
    ident = nc.dram_tensor("ident", [C, C], BF, kind="ExternalInput").ap()
    out = nc.dram_tensor("out", [NS, C, HW], BF, kind="ExternalOutput").ap()

    with tile.TileContext(nc) as tc:
        with ExitStack() as ctx:
            _body(tc, bass, mybir, xs, xts, t6ts, p2wt, w2t, ident, out, BF, F32, ctx, repeat, loop)
    nc.compile()
    _CACHE[("nc", repeat, loop)] = nc
    return nc


def host_inputs(x, p2w, p5w, conv_w):
    """Shard + prep per-core input maps from full inputs."""
    x = np.asarray(x, dtype=np.float32).reshape(N, C, HW)
    xbf = np.ascontiguousarray(x).astype(BF16NP)
    # x^T chunks: xt[n, l, j*C + c] = x[n, c, 112j + l]
    xt = np.ascontiguousarray(
        x.reshape(N, C, NCH, PCH).transpose(0, 3, 2, 1)
    ).reshape(N, PCH, NCH * C).astype(BF16NP)
    # t6 = x + roll(x, 1, rows); bf16(x) summed in f32, rounded once to bf16
    xb32 = xbf.astype(np.float32).reshape(N, C, H, W)
    t6 = (xb32 + np.roll(xb32, 1, axis=2)).reshape(N, C, HW)
    t6t = np.ascontiguousarray(
        t6.reshape(N, C, NCH, PCH).transpose(0, 3, 2, 1)
    ).reshape(N, PCH, NCH * C).astype(BF16NP)
    p2w_ = (np.asarray(p2w, dtype=np.float32)[0] * SCALE).reshape(C, 3, HW)
    # shifted d-slab: p2wt[l, d, j, c1] = p2w[c1, d, 112*(j-d+1)+l] (else 0)
    pw = p2w_.transpose(1, 2, 0).reshape(3, NCH, PCH, C)   # (d, j, l, c1)
    slab = np.zeros((3, NCH, PCH, C), np.float32)
    slab[0, 0 : NCH - 1] = pw[0, 1:NCH]
    slab[1] = pw[1]
    slab[2, 1:NCH] = pw[2, 0 : NCH - 1]
    p2wt = np.ascontiguousarray(
        slab.transpose(2, 0, 1, 3).reshape(PCH, 3 * NCH * C)
    ).astype(BF16NP)
    p5 = np.asarray(p5w, dtype=np.float32).reshape(C)
    cw = np.asarray(conv_w, dtype=np.float32)        # (C//G, C)
    W2 = cw[np.arange(C) % (C // G)] * p5[None, :]   # (c, c')
    w2t = np.ascontiguousarray(W2.T).astype(BF16NP)  # (c', c)
    ident = np.eye(C, dtype=BF16NP)
    in_maps = [
        {
            "xs": np.ascontiguousarray(xbf[i * NS : (i + 1) * NS]),
            "xts": np.ascontiguousarray(xt[i * NS : (i + 1) * NS]),
            "t6ts": np.ascontiguousarray(t6t[i * NS : (i + 1) * NS]),
            "p2wt": p2wt,
            "w2t": w2t,
            "ident": ident,
        }
        for i in range(NCORES)
    ]
    return in_maps


def _get_runner(repeat=1, loop=1):
    """Build (once) a persistent jitted shard_map executable over 8 cores."""
    if ("runner", repeat, loop) in _CACHE:
        return _CACHE[("runner", repeat, loop)]
    import jax
    from jax.sharding import Mesh, PartitionSpec
    from jax.experimental.shard_map import shard_map
    from concourse import bass2jax, mybir

    nc = build(repeat, loop)
    bass2jax.install_neuronx_cc_hook()

    partition_name = nc.partition_id_tensor.name if nc.partition_id_tensor else None
    in_names, out_names, out_avals, zero_outs = [], [], [], []
    for alloc in nc.m.functions[0].allocations:
        if not isinstance(alloc, mybir.MemoryLocationSet):
            continue
        name = alloc.memorylocations[0].name
        if alloc.kind == "ExternalInput":
            if name != partition_name:
                in_names.append(name)
        elif alloc.kind == "ExternalOutput":
            shape = tuple(alloc.tensor_shape)
            dtype = mybir.dt.np(alloc.dtype)
            out_avals.append(jax.core.ShapedArray(shape, dtype))
            zero_outs.append(np.zeros(shape, dtype))
            out_names.append(name)
    n_params = len(in_names)
    n_outs = len(out_avals)
    all_in_names = list(in_names) + list(out_names)
    if partition_name is not None:
        all_in_names.append(partition_name)
    donate = tuple(range(n_params, n_params + n_outs))

    def _body(*args):
        operands = list(args)
        if partition_name is not None:
            operands.append(bass2jax.partition_id_tensor())
        outs = bass2jax._bass_exec_p.bind(
            *operands,
            out_avals=tuple(out_avals),
            in_names=tuple(all_in_names),
            out_names=tuple(out_names),
            lowering_input_output_aliases=(),
            sim_require_finite=True,
            sim_require_nnan=True,
            nc=nc,
        )
        return tuple(outs)

    devices = jax.devices()[:NCORES]
    mesh = Mesh(np.asarray(devices), ("core",))
    in_specs = (PartitionSpec("core"),) * (n_params + n_outs)
    out_specs = (PartitionSpec("core"),) * n_outs
    sharded = jax.jit(
        shard_map(
            _body, mesh=mesh, in_specs=in_specs, out_specs=out_specs, check_rep=False
        ),
        donate_argnums=donate,
        keep_unused=True,
    )
    runner = {
        "fn": sharded,
        "in_names": in_names,
        "out_names": out_names,
        "out_avals": out_avals,
        "mesh": mesh,
        "n_params": n_params,
    }
    _CACHE[("runner", repeat, loop)] = runner
    return runner


def _concat_inputs(runner, in_maps):
    return [
        np.concatenate([np.asarray(m[name]) for m in in_maps], axis=0)
        for name in runner["in_names"]
    ]


def _zero_bufs(runner):
    return [
        np.zeros((NCORES * a.shape[0], *a.shape[1:]), a.dtype)
        for a in runner["out_avals"]
    ]


def run_fast(in_maps):
    """Execute via the cached jitted executable; returns list of per-core dicts."""
    runner = _get_runner()
    out_arrs = runner["fn"](*_concat_inputs(runner, in_maps), *_zero_bufs(runner))
    res = []
    for c in range(NCORES):
        res.append(
            {
                name: np.asarray(out_arrs[i]).reshape(
                    NCORES, *runner["out_avals"][i].shape
                )[c]
                for i, name in enumerate(runner["out_names"])
            }
        )
    return res


def run(in_maps, trace=False, **kw):
    from concourse.bass_utils import run_bass_kernel_spmd

    nc = build()
    return run_bass_kernel_spmd(nc, in_maps, list(range(NCORES)), trace=trace, **kw)


def bench_repeat(in_maps, R=8, reps=10, iters=8):
    """Per-kernel time from an R-times-unrolled program vs the 1x program."""
    import time

    import jax
    from jax.sharding import NamedSharding, PartitionSpec

    def timed(repeat):
        runner = _get_runner(repeat)
        sh = NamedSharding(runner["mesh"], PartitionSpec("core"))
        dev_in = [jax.device_put(a, sh) for a in _concat_inputs(runner, in_maps)]
        outs = [jax.device_put(z, sh) for z in _zero_bufs(runner)]
        jax.block_until_ready(dev_in)
        jax.block_until_ready(outs)
        outs = runner["fn"](*dev_in, *outs)  # warmup + first chain
        jax.block_until_ready(outs)
        ts = []
        for _ in range(reps):
            t0 = time.perf_counter()
            for _ in range(iters):
                outs = runner["fn"](*dev_in, *outs)
            jax.block_until_ready(outs)
            ts.append((time.perf_counter() - t0) / iters)
        return min(ts)

    t1 = timed(1)
    tR = timed(R)
    per = (tR - t1) / (R - 1)
    return per, t1, tR


def kernel(x, p2w, p5w, conv_w):
    in_maps = host_inputs(x, p2w, p5w, conv_w)
    res = run_fast(in_maps)
    outs = [np.asarray(res[i]["out"]) for i in range(NCORES)]
    return (
        np.concatenate(outs, axis=0).reshape(N, C, H, W).astype(np.float32)
    )


def bench_loop(in_maps, K1=2, K2=18, rounds=12, iters=4):
    """Per-kernel time from the slope between two loop-count programs of
    identical size. Interleaved rounds cancel drift."""
    import time
    import jax
    from jax.sharding import NamedSharding, PartitionSpec

    state = {}
    for K in (K1, K2):
        r = _get_runner(1, K)
        sh = NamedSharding(r["mesh"], PartitionSpec("core"))
        dev_in = [jax.device_put(a, sh) for a in _concat_inputs(r, in_maps)]
        outs = [jax.device_put(z, sh) for z in _zero_bufs(r)]
        jax.block_until_ready(dev_in)
        jax.block_until_ready(outs)
        outs = r["fn"](*dev_in, *outs)
        jax.block_until_ready(outs)
        state[K] = [r, dev_in, outs, 1e9]
    for _ in range(rounds):
        for K in (K1, K2):
            r, dev_in, outs, best = state[K]
            t0 = time.perf_counter()
            for _ in range(iters):
                outs = r["fn"](*dev_in, *outs)
            jax.block_until_ready(outs)
            dt = (time.perf_counter() - t0) / iters
            state[K][2] = outs
            state[K][3] = min(best, dt)
    t1, t2 = state[K1][3], state[K2][3]
    return (t2 - t1) / (K2 - K1), t1, t2
